# revision 8
# baseline (speedup 1.0000x reference)
"""HGTDetector (heterogeneous GNN) on 8 TRN2 NeuronCores.

Strategy:
- Nodes block-sharded: 6250 users + 12500 tweets per core (padded to 6272/12544).
- All relation/K/V 128x128 transforms are algebraically folded to node level:
    score(e)  = qrel[dst] . x_user[src]      (qrel = (Wk@arel)^T-scaled @ q[dst];
                                              bk term dropped: softmax shift-invariant)
    agg_v[d]  = (sum_e alpha_e x_user[src]) @ (Wv@mrel) + 1{deg>0} (bv@mrel)
  so only x_user [50000,128] is AllGathered; per-edge traffic is a single
  512B row gather per edge per pass.
- Edges bucketed by dst owner core and dst block (128 dsts); segment softmax +
  weighted aggregation done with one-hot scatter matmuls accumulating in PSUM.
- Everything else (encoders, gelu/skip/classify) is local feature-major matmul.
"""
import sys
sys.path.insert(0, '/opt/trn_rl_repo')
import numpy as np
import concourse.bass as bass
import concourse.bacc as bacc
import concourse.mybir as mybir
import concourse.tile as tile
from concourse.bass_utils import run_bass_kernel_spmd

P = 128
W = 8
N_USER, N_TWEET, D, E_EDGE = 50000, 100000, 128, 200000
U_LOC, T_LOC = N_USER // W, N_TWEET // W          # 6250, 12500
UB, TB = 49, 98                                   # dst blocks per core
U_PAD, T_PAD = UB * P, TB * P                     # 6272, 12544
f32 = mybir.dt.float32
i32 = mybir.dt.int32
AF = mybir.ActivationFunctionType
ALU = mybir.AluOpType
NEG = -1.0e30


def _bucket_edges(edges, n_dst_loc, nblocks, src_owner_pad, tfix=None):
    """Bucket edges by (dst core, dst block). Returns per-core swizzled arrays.

    edges: [2, E] int64 global (src always user). Returns (tfix, per_core dict).
    """
    src, dst = np.asarray(edges[0]), np.asarray(edges[1])
    core = dst // n_dst_loc
    ld = dst - core * n_dst_loc
    blk = ld // P
    per_core = []
    counts = np.zeros((W, nblocks), np.int64)
    for c in range(W):
        m = core == c
        s, l, b = src[m], ld[m], blk[m]
        order = np.argsort(b, kind='stable')
        per_core.append((s[order], l[order], b[order]))
        np.add.at(counts[c], blk[m], 1)
    need = int(np.ceil(counts.max() / P))
    tf = max(1, need if tfix is None else max(tfix, need))
    cap = tf * P
    out = []
    for c in range(W):
        s, l, b = per_core[c]
        n = nblocks * cap
        eidx = np.zeros(n, np.int32)
        qidx = np.zeros(n, np.int32)
        doff = np.zeros(n, np.float32)
        smask = np.full(n, NEG, np.float32)
        # positions: for each block, place its edges at blk*cap + rank
        startpos = np.searchsorted(b, np.arange(nblocks))
        rank = np.arange(len(b)) - startpos[b]
        pos = b * cap + rank
        eidx[pos] = ((s // U_LOC) * U_PAD + (s % U_LOC)).astype(np.int32)
        qidx[pos] = l.astype(np.int32)
        doff[pos] = (l % P).astype(np.float32)
        smask[pos] = 0.0
        nt = nblocks * tf
        out.append(dict(
            eidx=np.ascontiguousarray(eidx.reshape(nt, P).T),
            qidx=np.ascontiguousarray(qidx.reshape(nt, P).T),
            doff=np.ascontiguousarray(doff.reshape(nt, P).T),
            smask=np.ascontiguousarray(smask.reshape(nt, P).T)))
    return tf, out


def _pad_T(a, cols):
    """[n, k] -> transposed+padded [k, cols] f32 (feature-major)."""
    a = np.asarray(a, np.float32)
    out = np.zeros((a.shape[1], cols), np.float32)
    out[:, :a.shape[0]] = a.T
    return out


def _col(v, rows=P):
    v = np.asarray(v, np.float32).reshape(-1)
    out = np.zeros((rows, 1), np.float32)
    out[:len(v), 0] = v
    return out


def build_and_run(inp, want_trace=False, tmpdir=None):
    p = inp['params']
    g = lambda k: np.asarray(p[k], np.float32)
    isd = 1.0 / np.sqrt(D)

    # ---- host-folded weights ----
    Wc, bc, Wn, bn = g('Wc'), g('bc'), g('Wn'), g('bn')
    Wp, bp, Wpo, bpo = g('Wp'), g('bp'), g('Wpo'), g('bpo')
    Wt, bt = g('Wt'), g('bt')
    Wq_u, bq_u, Wq_t, bq_t = g('Wq_user'), g('bq_user'), g('Wq_tweet'), g('bq_tweet')
    Wk_u, Wv_u, bv_u = g('Wk_user'), g('Wv_user'), g('bv_user')
    W1T, W2, b2 = {}, {}, {}
    for e in ('follow', 'friend', 'post'):
        prel = float(np.asarray(p['prel_' + e]))
        W1T[e] = np.ascontiguousarray(((Wk_u @ g('arel_' + e)) * (prel * isd)).T)
        W2[e] = np.ascontiguousarray(Wv_u @ g('mrel_' + e))
        b2[e] = bv_u @ g('mrel_' + e)
    Wa_u, ba_u, Wa_t, ba_t = g('Wa_user'), g('ba_user'), g('Wa_tweet'), g('ba_tweet')
    sk_u = float(1.0 / (1.0 + np.exp(-float(np.asarray(p['skip_user'])))))
    sk_t = float(1.0 / (1.0 + np.exp(-float(np.asarray(p['skip_tweet'])))))
    Wc1, bc1, Wc2, bc2 = g('Wc1'), g('bc1'), g('Wc2'), g('bc2')

    # ---- shard node features (feature-major) ----
    cat, num, text = np.asarray(inp['cat_prop'], np.float32), np.asarray(inp['num_prop'], np.float32), np.asarray(inp['text'], np.float32)
    def _cn(c):
        out = np.zeros((64, U_PAD), np.float32)
        out[0:4] = _pad_T(cat[c*U_LOC:(c+1)*U_LOC], U_PAD)
        out[32:37] = _pad_T(num[c*U_LOC:(c+1)*U_LOC], U_PAD)
        return out
    cnT = [_cn(c) for c in range(W)]
    textT = [_pad_T(text[c*T_LOC:(c+1)*T_LOC], T_PAD) for c in range(W)]

    # ---- bucket edges ----
    TF_F, bF = _bucket_edges(inp['follow_edges'], U_LOC, UB, U_PAD)
    TF_G, bG = _bucket_edges(inp['friend_edges'], U_LOC, UB, U_PAD)
    TF_U = max(TF_F, TF_G)
    if TF_F != TF_U:
        TF_F, bF = _bucket_edges(inp['follow_edges'], U_LOC, UB, U_PAD, TF_U)
    if TF_G != TF_U:
        TF_G, bG = _bucket_edges(inp['friend_edges'], U_LOC, UB, U_PAD, TF_U)
    TF_P, bP = _bucket_edges(inp['post_edges'], T_LOC, TB, U_PAD)
    NT_F, NT_P = UB * TF_U, TB * TF_P

    IOTA = np.tile(np.arange(P, dtype=np.float32), (P, 1))
    IDENT = np.eye(P, dtype=np.float32)

    # ================= build program =================
    nc = bacc.Bacc("TRN2", target_bir_lowering=False, debug=False, num_devices=W)
    dt = lambda n, s, d=f32: nc.dram_tensor(n, s, d, kind="ExternalInput")
    t_cnT = dt("cnT", [64, U_PAD])
    t_textT = dt("textT", [768, T_PAD])
    t_iota, t_ident = dt("iota", [P, P]), dt("ident", [P, P])
    wnames = {}
    def wt(n, arr):
        arr = np.asarray(arr, np.float32)
        wnames[n] = arr
        return dt(n, list(arr.shape))
    t_Wc, t_Wn = wt("Wc", Wc), wt("Wn", Wn)
    t_bcn = wt("bcn", _col(np.concatenate([bc, bn])))
    t_Wp, t_bp = wt("Wp", Wp), wt("bp", _col(bp))
    t_Wpo, t_bpo = wt("Wpo", Wpo), wt("bpo", _col(bpo))
    t_Wt, t_bt = wt("Wtw", Wt), wt("btw", _col(bt))
    t_Wq_u, t_bq_u = wt("Wq_u", Wq_u), wt("bq_u", _col(bq_u))
    t_Wq_t, t_bq_t = wt("Wq_t", Wq_t), wt("bq_t", _col(bq_t))
    t_W1T = {e: wt("W1T_" + e, W1T[e]) for e in W1T}
    t_W2 = {e: wt("W2_" + e, W2[e]) for e in W2}
    t_b2 = {e: wt("b2_" + e, b2[e].reshape(1, P)) for e in b2}
    t_Wa_u, t_ba_u = wt("Wa_u", Wa_u), wt("ba_u", _col(ba_u))
    t_Wa_t, t_ba_t = wt("Wa_t", Wa_t), wt("ba_t", _col(ba_t))
    t_Wc1, t_bc1 = wt("Wc1", Wc1), wt("bc1", _col(bc1))
    t_Wc2, t_bc2 = wt("Wc2", Wc2), wt("bc2", _col(bc2))
    em = {}
    for nm, nt in (("F", NT_F), ("G", NT_F), ("Pp", NT_P)):
        em[nm] = dict(eidx=dt(nm + "_eidx", [P, nt], i32), qidx=dt(nm + "_qidx", [P, nt], i32),
                      doff=dt(nm + "_doff", [P, nt]), smask=dt(nm + "_smask", [P, nt]))
    out_u = nc.dram_tensor("out_user", [2, U_PAD], f32, kind="ExternalOutput")
    out_t = nc.dram_tensor("out_tweet", [2, T_PAD], f32, kind="ExternalOutput")

    xu_send = nc.dram_tensor("xu_send", [U_PAD, P], f32, kind="Internal")
    xu_all = nc.dram_tensor("xu_all", [W * U_PAD, P], f32, kind="Internal", addr_space="Shared")
    q_nm = {"follow": nc.dram_tensor("qF_nm", [U_PAD, P], f32, kind="Internal"),
            "friend": nc.dram_tensor("qG_nm", [U_PAD, P], f32, kind="Internal"),
            "post": nc.dram_tensor("qP_nm", [T_PAD, P], f32, kind="Internal")}

    with tile.TileContext(nc) as tc:
        with tc.tile_pool(name="const", bufs=1) as cpool, \
             tc.tile_pool(name="big", bufs=1) as bigp, \
             tc.tile_pool(name="wk", bufs=2) as wkp, \
             tc.tile_pool(name="ps", bufs=2, space="PSUM") as ps, \
             tc.tile_pool(name="psb", bufs=2, space="PSUM") as psb, \
             tc.tile_pool(name="pagg", bufs=2, space="PSUM") as pagg:

            def load_const(t, shape, dtype=f32):
                s = cpool.tile(shape, dtype, tag="c_" + t.name, name="c_" + t.name)
                nc.sync.dma_start(s[:], t.ap()[:])
                return s
            iota_sb = load_const(t_iota, [P, P])
            ident_sb = load_const(t_ident, [P, P])
            wsb = {}
            wn32_tile = cpool.tile([64, 32], f32, tag="c_Wn32")
            nc.sync.dma_start(wn32_tile[32:37, :], t_Wn.ap()[:])
            wsb["Wn32"] = wn32_tile
            wtw_tile = cpool.tile([P, 768], f32, tag="c_Wtw")
            wsb["Wtw"] = wtw_tile
            for k in range(6):
                nc.sync.dma_start(wsb["Wtw"][:, bass.ts(k, P)], t_Wt.ap()[bass.ts(k, P), :])
            for tn, arr in wnames.items():
                if tn == "Wtw":
                    continue
                wsb[tn] = load_const({"Wc": t_Wc, "Wn": t_Wn, "bcn": t_bcn, "Wp": t_Wp, "bp": t_bp,
                                      "Wpo": t_Wpo, "bpo": t_bpo, "Wtw": t_Wt, "btw": t_bt,
                                      "Wq_u": t_Wq_u, "bq_u": t_bq_u, "Wq_t": t_Wq_t, "bq_t": t_bq_t,
                                      "W1T_follow": t_W1T['follow'], "W1T_friend": t_W1T['friend'], "W1T_post": t_W1T['post'],
                                      "W2_follow": t_W2['follow'], "W2_friend": t_W2['friend'], "W2_post": t_W2['post'],
                                      "b2_follow": t_b2['follow'], "b2_friend": t_b2['friend'], "b2_post": t_b2['post'],
                                      "Wa_u": t_Wa_u, "ba_u": t_ba_u, "Wa_t": t_Wa_t, "ba_t": t_ba_t,
                                      "Wc1": t_Wc1, "bc1": t_bc1, "Wc2": t_Wc2, "bc2": t_bc2}[tn],
                                     list(arr.shape))
            xu_fm = bigp.tile([P, U_PAD], f32)
            xt_fm = bigp.tile([P, T_PAD], f32)

            # ---------- user encoder ----------
            for t in range(UB):
                sl = bass.ts(t, P)
                cn = wkp.tile([64, P], f32, tag="cn")
                nc.sync.dma_start(cn[0:4, :], t_cnT.ap()[0:4, sl])
                nc.sync.dma_start(cn[32:37, :], t_cnT.ap()[32:37, sl])
                h1p = ps.tile([64, P], f32, tag="enc", space="PSUM")
                nc.tensor.matmul(h1p[0:32, :], wsb["Wc"][:], cn[0:4, :], start=True, stop=True)
                nc.tensor.matmul(h1p[32:64, :], wsb["Wn32"][32:37, :], cn[32:37, :], start=True, stop=True)
                h1 = wkp.tile([64, P], f32, tag="h1")
                nc.scalar.activation(h1[:], h1p[:], AF.Lrelu, bias=wsb["bcn"][0:64, :], alpha=0.01)
                h2p = ps.tile([64, P], f32, tag="enc", space="PSUM")
                nc.tensor.matmul(h2p[:], wsb["Wp"][:], h1[:], start=True, stop=True)
                h2 = wkp.tile([64, P], f32, tag="h2")
                nc.scalar.activation(h2[:], h2p[:], AF.Lrelu, bias=wsb["bp"][0:64, :], alpha=0.01)
                xup = ps.tile([P, P], f32, tag="enc", space="PSUM")
                nc.tensor.matmul(xup[:], wsb["Wpo"][:], h2[:], start=True, stop=True)
                nc.scalar.activation(xu_fm[:, sl], xup[:], AF.Lrelu, bias=wsb["bpo"][:, :], alpha=0.01)
                # transpose to node-major + send for allgather
                xtp = ps.tile([P, P], f32, tag="encT", space="PSUM")
                nc.tensor.transpose(out=xtp[:], in_=xu_fm[:, sl], identity=ident_sb[:])
                xts = wkp.tile([P, P], f32, tag="xts")
                nc.vector.tensor_copy(xts[:], xtp[:])
                nc.sync.dma_start(xu_send.ap()[sl, :], xts[:])
            nc.gpsimd.collective_compute(
                "AllGather", ALU.bypass, replica_groups=[list(range(W))],
                ins=[xu_send.ap()[:]], outs=[xu_all.ap()[:]])

            # ---------- q_user + qrel follow/friend ----------
            for t in range(UB):
                sl = bass.ts(t, P)
                qp = ps.tile([P, P], f32, tag="enc", space="PSUM")
                nc.tensor.matmul(qp[:], wsb["Wq_u"][:], xu_fm[:, sl], start=True, stop=True)
                qfm = wkp.tile([P, P], f32, tag="qfm")
                nc.scalar.activation(qfm[:], qp[:], AF.Identity, bias=wsb["bq_u"][:, :])
                for e in ("follow", "friend"):
                    qnp = ps.tile([P, P], f32, tag="encT", space="PSUM")
                    nc.tensor.matmul(qnp[:], qfm[:], wsb["W1T_" + e][:], start=True, stop=True)
                    qns = wkp.tile([P, P], f32, tag="qns")
                    nc.vector.tensor_copy(qns[:], qnp[:])
                    nc.sync.dma_start(q_nm[e].ap()[sl, :], qns[:])

            # ---------- tweet encoder + q_tweet + qrel post ----------
            textT_r = t_textT.ap().rearrange("(a p) n -> a p n", p=P)
            for t in range(TB):
                sl = bass.ts(t, P)
                xtw = wkp.tile([P, 6 * P], f32, tag="txt")
                for k in range(6):
                    nc.sync.dma_start(xtw[:, bass.ts(k, P)], textT_r[k, :, sl])
                twp = ps.tile([P, P], f32, tag="enc", space="PSUM")
                for k in range(6):
                    nc.tensor.matmul(twp[:], wsb["Wtw"][:, bass.ts(k, P)], xtw[:, bass.ts(k, P)],
                                     start=(k == 0), stop=(k == 5))
                nc.scalar.activation(xt_fm[:, sl], twp[:], AF.Lrelu, bias=wsb["btw"][:, :], alpha=0.01)
                qp2 = ps.tile([P, P], f32, tag="enc", space="PSUM")
                nc.tensor.matmul(qp2[:], wsb["Wq_t"][:], xt_fm[:, sl], start=True, stop=True)
                qfm2 = wkp.tile([P, P], f32, tag="qfm")
                nc.scalar.activation(qfm2[:], qp2[:], AF.Identity, bias=wsb["bq_t"][:, :])
                qnp2 = ps.tile([P, P], f32, tag="encT", space="PSUM")
                nc.tensor.matmul(qnp2[:], qfm2[:], wsb["W1T_post"][:], start=True, stop=True)
                qns2 = wkp.tile([P, P], f32, tag="qns")
                nc.vector.tensor_copy(qns2[:], qnp2[:])
                nc.sync.dma_start(q_nm['post'].ap()[sl, :], qns2[:])

            # ---------- edge meta to SBUF ----------
            meta = {}
            for nm, nt in (("F", NT_F), ("G", NT_F), ("Pp", NT_P)):
                meta[nm] = {}
                for k2, d2 in (("eidx", i32), ("qidx", i32), ("doff", f32), ("smask", f32)):
                    s = cpool.tile([P, nt], d2, tag=f"m_{nm}_{k2}", name=f"m_{nm}_{k2}")
                    nc.sync.dma_start(s[:], em[nm][k2].ap()[:])
                    meta[nm][k2] = s
            scores = {nm: cpool.tile([P, nt], f32, tag=f"sc_{nm}", name=f"sc_{nm}")
                      for nm, nt in (("F", NT_F), ("G", NT_F), ("Pp", NT_P))}

            # ---------- pass 1: scores ----------
            def score_pass(nm, qsrc, nt):
                m = meta[nm]
                for k in range(nt):
                    qg = wkp.tile([P, P], f32, tag="qg")
                    nc.gpsimd.indirect_dma_start(
                        out=qg[:], out_offset=None, in_=qsrc.ap()[:],
                        in_offset=bass.IndirectOffsetOnAxis(ap=m["qidx"][:, k:k+1], axis=0))
                    xg = wkp.tile([P, P], f32, tag="xg")
                    nc.gpsimd.indirect_dma_start(
                        out=xg[:], out_offset=None, in_=xu_all.ap()[:],
                        in_offset=bass.IndirectOffsetOnAxis(ap=m["eidx"][:, k:k+1], axis=0))
                    pr = wkp.tile([P, P], f32, tag="pr")
                    nc.vector.tensor_tensor(out=pr[:], in0=qg[:], in1=xg[:], op=ALU.mult)
                    sc = wkp.tile([P, 1], f32, tag="sccol")
                    nc.vector.reduce_sum(out=sc[:], in_=pr[:], axis=mybir.AxisListType.X)
                    nc.vector.tensor_scalar(out=scores[nm][:, k:k+1], in0=sc[:],
                                            scalar1=m["smask"][:, k:k+1], scalar2=None, op0=ALU.add)
            score_pass("F", q_nm['follow'], NT_F)
            score_pass("G", q_nm['friend'], NT_F)
            score_pass("Pp", q_nm['post'], NT_P)

            # ---------- global max -> negmax column ----------
            mx = wkp.tile([P, 3], f32, tag="mx")
            nc.vector.reduce_max(out=mx[:, 0:1], in_=scores["F"][:], axis=mybir.AxisListType.X)
            nc.vector.reduce_max(out=mx[:, 1:2], in_=scores["G"][:], axis=mybir.AxisListType.X)
            nc.vector.reduce_max(out=mx[:, 2:3], in_=scores["Pp"][:], axis=mybir.AxisListType.X)
            mxc = wkp.tile([P, 1], f32, tag="mxc")
            nc.vector.reduce_max(out=mxc[:], in_=mx[:], axis=mybir.AxisListType.X)
            mxr_p = ps.tile([1, P], f32, tag="encT", space="PSUM")
            nc.tensor.transpose(out=mxr_p[:], in_=mxc[:], identity=ident_sb[:])
            mxr = wkp.tile([1, P], f32, tag="mxr")
            nc.vector.tensor_copy(mxr[:], mxr_p[:])
            m11 = wkp.tile([1, 1], f32, tag="m11")
            nc.vector.reduce_max(out=m11[:], in_=mxr[:], axis=mybir.AxisListType.X)
            nm11 = wkp.tile([1, 1], f32, tag="nm11")
            nc.vector.tensor_scalar(out=nm11[:], in0=m11[:], scalar1=-1.0, scalar2=None, op0=ALU.mult)
            ones_r = wkp.tile([1, P], f32, tag="ones")
            nc.vector.memset(ones_r[:], 1.0)
            ngp = ps.tile([P, 1], f32, tag="encT", space="PSUM")
            nc.tensor.matmul(ngp[:], ones_r[:], nm11[:], start=True, stop=True)
            negmax = cpool.tile([P, 1], f32, tag="negmax")
            nc.vector.tensor_copy(negmax[:], ngp[:])

            # ---------- pass 2 ----------
            def block_agg(nm, tf, b):
                """Process one dst block of one edge type -> (aggfm sbuf, indrow sbuf)."""
                m = meta[nm]
                ap_ = pagg.tile([P, P + 1], f32, tag="agg", space="PSUM")
                for j in range(tf):
                    k = b * tf + j
                    ev = wkp.tile([P, P + 1], f32, tag="ev")
                    nc.scalar.activation(ev[:, P:P+1], scores[nm][:, k:k+1], AF.Exp, bias=negmax[:, :])
                    xg = wkp.tile([P, P], f32, tag="xg2")
                    nc.gpsimd.indirect_dma_start(
                        out=xg[:], out_offset=None, in_=xu_all.ap()[:],
                        in_offset=bass.IndirectOffsetOnAxis(ap=m["eidx"][:, k:k+1], axis=0))
                    nc.vector.tensor_scalar(out=ev[:, 0:P], in0=xg[:], scalar1=ev[:, P:P+1],
                                            scalar2=None, op0=ALU.mult)
                    oh = wkp.tile([P, P], f32, tag="oh")
                    nc.vector.tensor_tensor(out=oh[:], in0=m["doff"][:, k:k+1].to_broadcast([P, P]),
                                            in1=iota_sb[:], op=ALU.is_equal)
                    nc.tensor.matmul(ap_[:], oh[:], ev[:], start=(j == 0), stop=(j == tf - 1))
                zc = wkp.tile([P, 1], f32, tag="zc")
                nc.vector.tensor_scalar(out=zc[:], in0=ap_[:, P:P+1], scalar1=1e-30, scalar2=None, op0=ALU.max)
                rec = wkp.tile([P, 1], f32, tag="rec")
                nc.vector.reciprocal(rec[:], zc[:])
                aggn = wkp.tile([P, P], f32, tag="aggn")
                nc.vector.tensor_scalar(out=aggn[:], in0=ap_[:, 0:P], scalar1=rec[:, :], scalar2=None, op0=ALU.mult)
                ind = wkp.tile([P, 1], f32, tag="ind")
                nc.vector.tensor_scalar(out=ind[:], in0=ap_[:, P:P+1], scalar1=0.0, scalar2=None, op0=ALU.is_gt)
                afp = psb.tile([P, P], f32, tag="px", space="PSUM")
                nc.tensor.transpose(out=afp[:], in_=aggn[:], identity=ident_sb[:])
                aggfm = wkp.tile([P, P], f32, tag="aggfm")
                nc.vector.tensor_copy(aggfm[:], afp[:])
                irp = psb.tile([1, P], f32, tag="px", space="PSUM")
                nc.tensor.transpose(out=irp[:], in_=ind[:], identity=ident_sb[:])
                indrow = wkp.tile([1, P], f32, tag="indrow")
                nc.vector.tensor_copy(indrow[:], irp[:])
                return aggfm, indrow

            def node_out(b, vp, x_fm, Wa, ba, sk, out_hbm):
                sl = bass.ts(b, P)
                gl = wkp.tile([P, P], f32, tag="gl")
                nc.scalar.activation(gl[:], vp[:], AF.Gelu)
                op_ = psb.tile([P, P], f32, tag="px", space="PSUM")
                nc.tensor.matmul(op_[:], wsb[Wa][:], gl[:], start=True, stop=True)
                t1 = wkp.tile([P, P], f32, tag="t1")
                nc.vector.tensor_scalar(out=t1[:], in0=op_[:], scalar1=wsb[ba][:, :], scalar2=sk,
                                        op0=ALU.add, op1=ALU.mult)
                t2 = wkp.tile([P, P], f32, tag="t2")
                nc.vector.tensor_scalar(out=t2[:], in0=x_fm[:, sl], scalar1=1.0 - sk, scalar2=None, op0=ALU.mult)
                bl = wkp.tile([P, P], f32, tag="bl")
                nc.vector.tensor_tensor(out=bl[:], in0=t1[:], in1=t2[:], op=ALU.add)
                hp = psb.tile([P, P], f32, tag="px", space="PSUM")
                nc.tensor.matmul(hp[:], wsb["Wc1"][:], bl[:], start=True, stop=True)
                hs = wkp.tile([P, P], f32, tag="hs")
                nc.scalar.activation(hs[:], hp[:], AF.Lrelu, bias=wsb["bc1"][:, :], alpha=0.01)
                lp = psb.tile([2, P], f32, tag="px", space="PSUM")
                nc.tensor.matmul(lp[:], wsb["Wc2"][:], hs[:], start=True, stop=True)
                lo = wkp.tile([2, P], f32, tag="lo")
                nc.scalar.activation(lo[:], lp[:], AF.Identity, bias=wsb["bc2"][0:2, :])
                nc.sync.dma_start(out_hbm.ap()[:, sl], lo[:])

            for b in range(UB):
                aF, iF = block_agg("F", TF_U, b)
                aG, iG = block_agg("G", TF_U, b)
                vp = psb.tile([P, P], f32, tag="px", space="PSUM")
                nc.tensor.matmul(vp[:], wsb["W2_follow"][:], aF[:], start=True, stop=False)
                nc.tensor.matmul(vp[:], wsb["W2_friend"][:], aG[:], start=False, stop=False)
                nc.tensor.matmul(vp[:], wsb["b2_follow"][:], iF[:], start=False, stop=False)
                nc.tensor.matmul(vp[:], wsb["b2_friend"][:], iG[:], start=False, stop=True)
                node_out(b, vp, xu_fm, "Wa_u", "ba_u", sk_u, out_u)
            for b in range(TB):
                aP, iP = block_agg("Pp", TF_P, b)
                vp2 = psb.tile([P, P], f32, tag="px", space="PSUM")
                nc.tensor.matmul(vp2[:], wsb["W2_post"][:], aP[:], start=True, stop=False)
                nc.tensor.matmul(vp2[:], wsb["b2_post"][:], iP[:], start=False, stop=True)
                node_out(b, vp2, xt_fm, "Wa_t", "ba_t", sk_t, out_t)

    if not nc.is_finalized():
        nc.finalize()

    in_maps = []
    for c in range(W):
        m = dict(cnT=cnT[c], textT=textT[c], iota=IOTA, ident=IDENT)
        m.update(wnames)
        for nm, bb in (("F", bF), ("G", bG), ("Pp", bP)):
            for k2 in ("eidx", "qidx", "doff", "smask"):
                m[f"{nm}_{k2}"] = bb[c][k2]
        in_maps.append(m)
    res = run_bass_kernel_spmd(nc, in_maps, core_ids=list(range(W)), trace=want_trace, tmpdir=tmpdir)
    hu = np.concatenate([res.results[c]["out_user"][:, :U_LOC].T for c in range(W)], axis=0)
    ht = np.concatenate([res.results[c]["out_tweet"][:, :T_LOC].T for c in range(W)], axis=0)
    return (np.ascontiguousarray(hu), np.ascontiguousarray(ht)), res


def kernel(**inputs):
    out, _ = build_and_run(inputs)
    return out
